# revision 7
# baseline (speedup 1.0000x reference)
"""TransformerConv GNN (CircuitGNN) on 8 Trainium2 NeuronCores.

Strategy:
 - Shard graphs across 8 cores at graph boundaries (pooling stays local).
 - Per core, nodes are padded to P_N rows and grouped into 128-node blocks.
 - Edges are assigned to the core owning their dst, sorted by dst, grouped by
   dst block, split into 128-edge tiles.
 - Blocks are REORDERED per core (descending tile count) so that a single SPMD
   program with per-position tile counts T_pos = max over cores fits every
   core; shorter cores run masked pad tiles.
 - Per layer: node phase computes q/skip per block (dense matmuls, fp32r),
   edge tiles gather h[src] rows from a replicated full-h table via indirect
   DMA, compute k/v and attention on-chip, and scatter per-dst sums via
   one-hot matmuls into PSUM.  Softmax is computed without the segment-max
   shift (alpha is bounded, mathematically identical).
 - After layers 0..3 an AllGather replicates the new h to every core.
 - Layer 4 feeds a fused global-mean-pool (one-hot matmul), then the global
   MLP + regressor head run per core on its 64 graphs in transposed layout.
"""

import math
import numpy as np

N_CORES = 8
DK = 32  # channels per head (sqrt scaling)


# --------------------------------------------------------------------------
# host-side planning
# --------------------------------------------------------------------------

def _plan(x, edge_index, batch, n_graphs):
    N = x.shape[0]
    src_g = np.asarray(edge_index[0], np.int64)
    dst_g = np.asarray(edge_index[1], np.int64)
    gpc = n_graphs // N_CORES
    # node range per core (batch is sorted)
    gstart = np.searchsorted(batch, np.arange(n_graphs + 1))
    bounds = gstart[np.arange(0, n_graphs + 1, gpc)]          # [9]
    Nc = np.diff(bounds)
    P_N = max(128, int(math.ceil(Nc.max() / 128.0)) * 128)
    NB = P_N // 128

    owner = np.searchsorted(bounds, np.arange(N), side="right") - 1
    loc = np.arange(N) - bounds[owner]

    cores = []
    profiles = np.zeros((N_CORES, NB), np.int64)
    perms = []
    for c in range(N_CORES):
        em = owner[dst_g] == c
        es = src_g[em]
        ed = dst_g[em] - bounds[c]
        order = np.argsort(ed, kind="stable")
        es, ed = es[order], ed[order]
        blk = ed // 128
        ecnt = np.bincount(blk, minlength=NB)
        tcnt = -(-ecnt // 128)
        perm = np.argsort(-tcnt, kind="stable")               # pos -> orig blk
        perms.append(perm)
        profiles[c] = tcnt[perm]
        cores.append((es, ed, blk, ecnt, tcnt))

    # new node numbering: node in orig block b of core c sits at block
    # invperm[b] in the new order
    new_gid = np.empty(N, np.int64)
    invperms = []
    for c in range(N_CORES):
        invperm = np.empty(NB, np.int64)
        invperm[perms[c]] = np.arange(NB)
        invperms.append(invperm)
        m = owner == c
        lc = loc[m]
        new_gid[m] = c * P_N + invperm[lc // 128] * 128 + (lc % 128)

    T_pos = profiles.max(axis=0)                              # [NB]
    T_total = int(T_pos.sum())

    # per-core per-tile metadata
    metas = []
    for c in range(N_CORES):
        es, ed, blk, ecnt, tcnt = cores[c]
        estart = np.concatenate([[0], np.cumsum(ecnt)])
        srcs = np.zeros((T_total, 128), np.int32)
        dm = np.zeros((T_total, 128, 2), np.float32)
        t0 = 0
        for pos in range(NB):
            b = perms[c][pos]
            k = int(ecnt[b])
            base = int(estart[b])
            for t in range(int(T_pos[pos])):
                lo = t * 128
                n = max(0, min(128, k - lo))
                if n > 0:
                    sl = slice(base + lo, base + lo + n)
                    srcs[t0 + t, :n] = new_gid[es[sl]]
                    dm[t0 + t, :n, 0] = (ed[sl] % 128).astype(np.float32)
                    dm[t0 + t, :n, 1] = 1.0
            t0 += int(T_pos[pos])
        metas.append((srcs.reshape(-1, 1), dm.reshape(-1, 2)))

    return dict(P_N=P_N, NB=NB, bounds=bounds, Nc=Nc, owner=owner,
                perms=perms, invperms=invperms, T_pos=T_pos,
                T_total=T_total, metas=metas, gpc=gpc, gstart=gstart)


def _per_core_tables(plan, x_pad, batch, global_features, n_graphs):
    """xl [P_N, D0], gpool [NB*128, gpc], gT [n_glob, gpc] per core."""
    P_N, NB, gpc = plan["P_N"], plan["NB"], plan["gpc"]
    bounds, perms = plan["bounds"], plan["perms"]
    gstart = plan["gstart"]
    D0 = x_pad.shape[1]
    cnt = np.diff(gstart).astype(np.float32)
    xls, gpools, gts = [], [], []
    for c in range(N_CORES):
        n0, n1 = int(bounds[c]), int(bounds[c + 1])
        NcL = n1 - n0
        xl = np.zeros((P_N, D0), np.float32)
        gp = np.zeros((NB, 128, gpc), np.float32)
        for pos in range(NB):
            b = int(perms[c][pos])
            lo, hi = b * 128, min(b * 128 + 128, NcL)
            if lo < NcL:
                n = hi - lo
                xl[pos * 128:pos * 128 + n] = x_pad[n0 + lo:n0 + hi]
                gl = batch[n0 + lo:n0 + hi] - c * gpc
                w = 1.0 / np.maximum(cnt[c * gpc + gl], 1.0)
                gp[pos, np.arange(n), gl] = w
        xls.append(xl)
        gpools.append(gp.reshape(-1, gpc))
        gts.append(np.ascontiguousarray(
            global_features[c * gpc:(c + 1) * gpc].T.astype(np.float32)))
    return xls, gpools, gts


# --------------------------------------------------------------------------
# device program
# --------------------------------------------------------------------------

def _build(plan, shapes):
    import concourse.bacc as bacc
    import concourse.bass as bass
    import concourse.mybir as mybir
    import concourse.tile as tile
    from concourse.masks import make_identity

    f32 = mybir.dt.float32
    f32r = mybir.dt.float32r
    i32 = mybir.dt.int32
    AF = mybir.ActivationFunctionType
    OP = mybir.AluOpType

    P_N, NB, T_pos, T_total = plan["P_N"], plan["NB"], plan["T_pos"], plan["T_total"]
    gpc = plan["gpc"]
    L = shapes["layers"]            # number of conv layers
    D0 = shapes["D0"]               # padded input feature dim (128)
    HC = shapes["HC"]               # 256
    NG16 = shapes["NG16"]           # mlp hidden (16)
    REGIN = HC + NG16

    nc = bacc.Bacc("TRN2", target_bir_lowering=False, debug=False,
                   enable_asserts=False, num_devices=N_CORES,
                   num_swdge_queues=4)

    def din(layer):
        return D0 if layer == 0 else HC

    # ---- dram tensors -----------------------------------------------------
    xt = nc.dram_tensor("xt", [N_CORES * P_N, D0], f32, kind="ExternalInput").ap()
    xl = nc.dram_tensor("xl", [P_N, D0], f32, kind="ExternalInput").ap()
    srcs = nc.dram_tensor("srcs", [T_total * 128, 1], i32, kind="ExternalInput").ap()
    dmt = nc.dram_tensor("dm", [T_total * 128, 2], f32, kind="ExternalInput").ap()
    gpool = nc.dram_tensor("gpool", [NB * 128, gpc], f32, kind="ExternalInput").ap()
    gt_in = nc.dram_tensor("gt", [8, gpc], f32, kind="ExternalInput").ap()
    iotar = nc.dram_tensor("iotar", [128, 128], f32, kind="ExternalInput").ap()
    iotac = nc.dram_tensor("iotac", [128, 1], f32, kind="ExternalInput").ap()
    wqs, wkv, bqb, bkb, bvb, bsb = [], [], [], [], [], []
    for l in range(L):
        kc = din(l) // 128
        wqs.append(nc.dram_tensor(f"wqs{l}", [128, kc * 2 * HC], f32, kind="ExternalInput").ap())
        wkv.append(nc.dram_tensor(f"wkv{l}", [128, kc * 2 * HC], f32, kind="ExternalInput").ap())
        bqb.append(nc.dram_tensor(f"bq{l}", [128, HC], f32, kind="ExternalInput").ap())
        bkb.append(nc.dram_tensor(f"bk{l}", [128, HC], f32, kind="ExternalInput").ap())
        bvb.append(nc.dram_tensor(f"bv{l}", [128, HC], f32, kind="ExternalInput").ap())
        bsb.append(nc.dram_tensor(f"bs{l}", [128, HC], f32, kind="ExternalInput").ap())
    wm = []
    for i, shp in enumerate([[8, NG16], [NG16, NG16], [NG16, NG16]]):
        wm.append(nc.dram_tensor(f"wm{i}", shp, f32, kind="ExternalInput").ap())
    bm = [nc.dram_tensor(f"bm{i}", [NG16, 1], f32, kind="ExternalInput").ap()
          for i in range(3)]
    wr = []
    for i, shp in enumerate([[REGIN, NG16], [NG16, NG16], [NG16, 1]]):
        wr.append(nc.dram_tensor(f"wr{i}", shp, f32, kind="ExternalInput").ap())
    br = [nc.dram_tensor("br0", [NG16, 1], f32, kind="ExternalInput").ap(),
          nc.dram_tensor("br1", [NG16, 1], f32, kind="ExternalInput").ap(),
          nc.dram_tensor("br2", [1, 1], f32, kind="ExternalInput").ap()]
    y = nc.dram_tensor("y", [1, gpc], f32, kind="ExternalOutput").ap()

    hloc = [nc.dram_tensor(f"hloc{l}", [P_N, HC], f32) for l in range(1, L)]
    hfull = [nc.dram_tensor(f"hfull{l}", [N_CORES * P_N, HC], f32,
                            addr_space="Shared") for l in range(1, L)]

    inv_sqrt_dk = 1.0 / math.sqrt(DK)

    with tile.TileContext(nc) as tc:
        with tc.tile_pool(name="const", bufs=1) as cp, \
             tc.tile_pool(name="wpool", bufs=2) as wp, \
             tc.tile_pool(name="nodep", bufs=3) as np_, \
             tc.tile_pool(name="edgep", bufs=4) as ep, \
             tc.tile_pool(name="psA", bufs=1, space="PSUM") as psA, \
             tc.tile_pool(name="psB", bufs=2, space="PSUM") as psB, \
             tc.tile_pool(name="psC", bufs=1, space="PSUM") as psC, \
             tc.tile_pool(name="headp", bufs=1) as hp:

            ident = cp.tile([128, 128], f32)
            make_identity(nc, ident[:])
            ident_r = cp.tile([128, 128], f32r)
            nc.vector.tensor_copy(ident_r[:], ident[:])
            iotar_s = cp.tile([128, 128], f32)
            nc.sync.dma_start(iotar_s[:], iotar[:])
            iotac_s = cp.tile([128, 1], f32)
            nc.sync.dma_start(iotac_s[:], iotac[:])

            pool_acc = hp.tile([gpc, HC], f32)
            nc.vector.memset(pool_acc[:], 0.0)

            for l in range(L):
                D = din(l)
                KC = D // 128
                htab = xt if l == 0 else hfull[l - 1][:]
                hl = xl if l == 0 else hloc[l - 1][:]

                wqs_s = wp.tile([128, KC * 2 * HC], f32r, tag="wqs")
                wkv_s = wp.tile([128, KC * 2 * HC], f32r, tag="wkv")
                wtmp = wp.tile([128, KC * 2 * HC], f32, tag="wtmp")
                nc.sync.dma_start(wtmp[:], wqs[l][:])
                nc.vector.tensor_copy(wqs_s[:], wtmp[:])
                wtmp2 = wp.tile([128, KC * 2 * HC], f32, tag="wtmp")
                nc.sync.dma_start(wtmp2[:], wkv[l][:])
                nc.vector.tensor_copy(wkv_s[:], wtmp2[:])
                bq_s = wp.tile([128, HC], f32, tag="bq")
                nc.sync.dma_start(bq_s[:], bqb[l][:])
                bk_s = wp.tile([128, HC], f32, tag="bk")
                nc.sync.dma_start(bk_s[:], bkb[l][:])
                bv_s = wp.tile([128, HC], f32, tag="bv")
                nc.sync.dma_start(bv_s[:], bvb[l][:])
                bs_s = wp.tile([128, HC], f32, tag="bs")
                nc.sync.dma_start(bs_s[:], bsb[l][:])

                gtile0 = 0
                for pos in range(NB):
                    # ---------------- node phase ----------------
                    hb = np_.tile([128, D], f32, tag="hb")
                    nc.sync.dma_start(hb[:], hl[pos * 128:(pos + 1) * 128, :])
                    hbT = np_.tile([128, KC * 128], f32r, tag="hbT")
                    for kc in range(KC):
                        tp = psB.tile([128, 128], f32, tag="tp", space="PSUM")
                        nc.tensor.transpose(tp[:], hb[:, kc * 128:(kc + 1) * 128], ident[:])
                        nc.scalar.copy(hbT[:, kc * 128:(kc + 1) * 128], tp[:])
                    qs_ps = psA.tile([128, 2 * HC], f32, tag="qs", space="PSUM")
                    for kc in range(KC):
                        nc.tensor.matmul(qs_ps[:], hbT[:, kc * 128:(kc + 1) * 128],
                                         wqs_s[:, kc * 2 * HC:(kc + 1) * 2 * HC],
                                         start=(kc == 0), stop=(kc == KC - 1))
                    q_s = np_.tile([128, HC], f32r, tag="q_s")
                    nc.vector.tensor_add(q_s[:], qs_ps[:, 0:HC], bq_s[:])

                    # ---------------- edge tiles ----------------
                    TP = int(T_pos[pos])
                    agg = None
                    if TP > 0:
                        agg = psA.tile([128, 8 + HC], f32, tag="agg", space="PSUM")
                    for t in range(TP):
                        gt_i = gtile0 + t
                        mi = ep.tile([128, 1], i32, tag="mi")
                        nc.sync.dma_start(mi[:], srcs[gt_i * 128:(gt_i + 1) * 128, :])
                        mf = ep.tile([128, 2], f32, tag="mf")
                        nc.sync.dma_start(mf[:], dmt[gt_i * 128:(gt_i + 1) * 128, :])
                        gat = ep.tile([128, D], f32, tag="gat")
                        gd = nc.gpsimd.indirect_dma_start(
                            out=gat[:], out_offset=None, in_=htab,
                            in_offset=bass.IndirectOffsetOnAxis(ap=mi[:, :1], axis=0))
                        qn = gt_i % 4
                        if qn:
                            gd.ins.queue = f"qPoolDynamic{qn}"
                        gT = ep.tile([128, KC * 128], f32r, tag="gT")
                        for kc in range(KC):
                            tp = psB.tile([128, 128], f32, tag="tp", space="PSUM")
                            nc.tensor.transpose(tp[:], gat[:, kc * 128:(kc + 1) * 128],
                                                ident[:])
                            nc.scalar.copy(gT[:, kc * 128:(kc + 1) * 128], tp[:])
                        kv_ps = psB.tile([128, 2 * HC], f32, tag="kv", space="PSUM")
                        for kc in range(KC):
                            nc.tensor.matmul(kv_ps[:], gT[:, kc * 128:(kc + 1) * 128],
                                             wkv_s[:, kc * 2 * HC:(kc + 1) * 2 * HC],
                                             start=(kc == 0), stop=(kc == KC - 1))
                        ktile = ep.tile([128, HC], f32, tag="ktile")
                        nc.vector.tensor_add(ktile[:], kv_ps[:, 0:HC], bk_s[:])
                        # one-hot matrices from dst_rel
                        dstT = psB.tile([128, 128], f32, tag="tp", space="PSUM")
                        nc.tensor.transpose(dstT[:],
                                            mf[:, 0:1].to_broadcast([128, 128]),
                                            ident[:])
                        S_T = ep.tile([128, 128], f32r, tag="S_T")
                        nc.vector.tensor_tensor(
                            out=S_T[:], in0=dstT[:],
                            in1=iotac_s[:, 0:1].to_broadcast([128, 128]),
                            op=OP.is_equal)
                        S = ep.tile([128, 128], f32r, tag="S")
                        nc.vector.tensor_tensor(
                            out=S[:], in0=mf[:, 0:1].to_broadcast([128, 128]),
                            in1=iotar_s[:], op=OP.is_equal)
                        qdst = psC.tile([128, HC], f32, tag="qdst", space="PSUM")
                        nc.tensor.matmul(qdst[:], S_T[:], q_s[:], start=True, stop=True)
                        prod = ep.tile([128, HC], f32, tag="prod")
                        nc.vector.tensor_tensor(out=prod[:], in0=qdst[:], in1=ktile[:],
                                                op=OP.mult)
                        alpha = ep.tile([128, 8], f32, tag="alpha")
                        nc.vector.tensor_reduce(
                            out=alpha[:], in_=prod[:].rearrange("p (h c) -> p h c", c=DK),
                            axis=mybir.AxisListType.X, op=OP.add)
                        et = ep.tile([128, 8], f32, tag="et")
                        nc.scalar.activation(et[:], alpha[:], AF.Exp,
                                             scale=inv_sqrt_dk)
                        rhs_t = ep.tile([128, 8 + HC], f32r, tag="rhs_t")
                        nc.vector.tensor_tensor(
                            out=rhs_t[:, 0:8], in0=et[:],
                            in1=mf[:, 1:2].to_broadcast([128, 8]), op=OP.mult)
                        nc.vector.tensor_tensor(
                            out=rhs_t[:, 8:8 + HC].rearrange("p (h c) -> p h c", c=DK),
                            in0=rhs_t[:, 0:8].to_broadcast([128, 8, DK]),
                            in1=kv_ps[:, HC:2 * HC].rearrange("p (h c) -> p h c", c=DK),
                            op=OP.mult)
                        nc.tensor.matmul(agg[:], S[:], rhs_t[:],
                                         start=(t == 0), stop=(t == TP - 1))
                    gtile0 += TP

                    # ---------------- epilogue ----------------
                    hout_dt = f32r if l == L - 1 else f32
                    hout = np_.tile([128, HC], hout_dt, tag="hout")
                    if TP > 0:
                        d1 = np_.tile([128, 8], f32, tag="d1")
                        nc.vector.tensor_scalar_add(d1[:], agg[:, 0:8], 1e-16)
                        rec = np_.tile([128, 8], f32, tag="rec")
                        nc.vector.reciprocal(rec[:], d1[:])
                        bvd = np_.tile([128, HC], f32, tag="bvd")
                        nc.vector.tensor_tensor(
                            out=bvd[:].rearrange("p (h c) -> p h c", c=DK),
                            in0=bv_s[:].rearrange("p (h c) -> p h c", c=DK),
                            in1=agg[:, 0:8].to_broadcast([128, 8, DK]), op=OP.mult)
                        p2 = np_.tile([128, HC], f32, tag="p2")
                        nc.vector.tensor_add(p2[:], agg[:, 8:8 + HC], bvd[:])
                        u = np_.tile([128, HC], f32, tag="u")
                        nc.vector.tensor_tensor(
                            out=u[:].rearrange("p (h c) -> p h c", c=DK),
                            in0=p2[:].rearrange("p (h c) -> p h c", c=DK),
                            in1=rec[:].to_broadcast([128, 8, DK]), op=OP.mult)
                        w1 = np_.tile([128, HC], f32, tag="w1")
                        nc.vector.tensor_add(w1[:], u[:], qs_ps[:, HC:2 * HC])
                        w2 = np_.tile([128, HC], f32, tag="w2")
                        nc.vector.tensor_add(w2[:], w1[:], bs_s[:])
                    else:
                        w2 = np_.tile([128, HC], f32, tag="w2")
                        nc.vector.tensor_add(w2[:], qs_ps[:, HC:2 * HC], bs_s[:])
                    nc.scalar.activation(hout[:], w2[:], AF.Relu)

                    if l < L - 1:
                        nc.sync.dma_start(hloc[l][pos * 128:(pos + 1) * 128, :], hout[:])
                    else:
                        gps = np_.tile([128, gpc], f32, tag="gps")
                        nc.sync.dma_start(gps[:], gpool[pos * 128:(pos + 1) * 128, :])
                        gpr = np_.tile([128, gpc], f32r, tag="gpr")
                        nc.vector.tensor_copy(gpr[:], gps[:])
                        pl_ps = psC.tile([gpc, HC], f32, tag="qdst", space="PSUM")
                        nc.tensor.matmul(pl_ps[:], gpr[:], hout[:], start=True, stop=True)
                        nc.vector.tensor_add(pool_acc[:], pool_acc[:], pl_ps[:])

                if l < L - 1:
                    nc.gpsimd.collective_compute(
                        "AllGather", mybir.AluOpType.bypass,
                        replica_groups=[list(range(N_CORES))],
                        ins=[hloc[l][:].opt()],
                        outs=[hfull[l][:].opt()],
                    )

            # ------------- head: pool -> concat -> regressor ----------------
            pool_r = hp.tile([gpc, HC], f32r)
            nc.vector.tensor_copy(pool_r[:], pool_acc[:])
            poolT = hp.tile([128, 2 * gpc], f32r)
            for kc in range(2):
                tp = psB.tile([128, 128], f32r, tag="tp", space="PSUM")
                nc.tensor.transpose(tp[:, 0:gpc],
                                    pool_r[:, kc * 128:(kc + 1) * 128],
                                    ident_r[0:gpc, 0:gpc])
                nc.scalar.copy(poolT[:, kc * gpc:(kc + 1) * gpc], tp[:, 0:gpc])

            gt_s = hp.tile([8, gpc], f32r)
            gtmp = hp.tile([8, gpc], f32)
            nc.sync.dma_start(gtmp[:], gt_in[:])
            nc.vector.tensor_copy(gt_s[:], gtmp[:])

            def load_small(ap_in, p, q_, tagn):
                tt = hp.tile([p, q_], f32, tag=tagn + "f")
                nc.sync.dma_start(tt[:], ap_in)
                rr = hp.tile([p, q_], f32r, tag=tagn)
                nc.vector.tensor_copy(rr[:], tt[:])
                return rr

            cur = gt_s
            for i in range(3):
                wmi = load_small(wm[i][:], [8, NG16, NG16][i], NG16, f"wm{i}")
                bmi = hp.tile([NG16, 1], f32, tag=f"bm{i}")
                nc.sync.dma_start(bmi[:], bm[i][:])
                zz = psC.tile([NG16, gpc], f32, tag="qdst", space="PSUM")
                nc.tensor.matmul(zz[:], wmi[:], cur[:], start=True, stop=True)
                nxt = hp.tile([NG16, gpc], f32r, tag=f"m{i}")
                nc.scalar.activation(nxt[:], zz[:], AF.Relu, bias=bmi[:, 0:1])
                cur = nxt

            wr0a = load_small(wr[0][0:128, :], 128, NG16, "wr0a")
            wr0b = load_small(wr[0][128:256, :], 128, NG16, "wr0b")
            wr0c = load_small(wr[0][256:REGIN, :], NG16, NG16, "wr0c")
            z1 = psC.tile([NG16, gpc], f32, tag="qdst", space="PSUM")
            nc.tensor.matmul(z1[:], wr0a[:], poolT[:, 0:gpc], start=True, stop=False)
            nc.tensor.matmul(z1[:], wr0b[:], poolT[:, gpc:2 * gpc], start=False, stop=False)
            nc.tensor.matmul(z1[:], wr0c[:], cur[:], start=False, stop=True)
            br0_s = hp.tile([NG16, 1], f32)
            nc.sync.dma_start(br0_s[:], br[0][:])
            z1s = hp.tile([NG16, gpc], f32r)
            nc.scalar.activation(z1s[:], z1[:], AF.Relu, bias=br0_s[:, 0:1])

            wr1s = load_small(wr[1][:], NG16, NG16, "wr1")
            z2 = psC.tile([NG16, gpc], f32, tag="qdst", space="PSUM")
            nc.tensor.matmul(z2[:], wr1s[:], z1s[:], start=True, stop=True)
            br1_s = hp.tile([NG16, 1], f32)
            nc.sync.dma_start(br1_s[:], br[1][:])
            z2s = hp.tile([NG16, gpc], f32r)
            nc.scalar.activation(z2s[:], z2[:], AF.Relu, bias=br1_s[:, 0:1])

            wr2s = load_small(wr[2][:], NG16, 1, "wr2")
            z3 = psC.tile([1, gpc], f32, tag="qdst", space="PSUM")
            nc.tensor.matmul(z3[:], wr2s[:], z2s[:], start=True, stop=True)
            br2_s = hp.tile([1, 1], f32)
            nc.sync.dma_start(br2_s[:], br[2][:])
            ys = hp.tile([1, gpc], f32)
            nc.scalar.activation(ys[:], z3[:], AF.Identity, bias=br2_s[:, 0:1])
            nc.sync.dma_start(y[:], ys[:])

    nc.compile()
    return nc


# --------------------------------------------------------------------------
# entry point
# --------------------------------------------------------------------------

_LAST = {}


def kernel(x, edge_index, batch, global_features, conv_params, mlp_params,
           reg_params):
    from concourse import bass_utils
    from concourse.bass_interp import get_hw_module

    x = np.asarray(x, np.float32)
    edge_index = np.asarray(edge_index)
    batch_np = np.asarray(batch, np.int64)
    global_features = np.asarray(global_features, np.float32)
    conv_params = [{k: np.asarray(v, np.float32) for k, v in p.items()}
                   for p in conv_params]
    mlp_params = [(np.asarray(W, np.float32), np.asarray(b, np.float32))
                  for W, b in mlp_params]
    reg_params = [(np.asarray(W, np.float32), np.asarray(b, np.float32))
                  for W, b in reg_params]

    n_graphs = global_features.shape[0]
    HC = conv_params[0]["Wq"].shape[1]
    L = len(conv_params)
    D0 = 128
    NG16 = mlp_params[0][0].shape[1]
    gpc = n_graphs // N_CORES

    plan = _plan(x, edge_index, batch_np, n_graphs)
    P_N, NB, T_total = plan["P_N"], plan["NB"], plan["T_total"]

    x_pad = np.zeros((x.shape[0], D0), np.float32)
    x_pad[:, :x.shape[1]] = x
    xls, gpools, gts = _per_core_tables(plan, x_pad, batch_np, global_features,
                                        n_graphs)
    xt = np.concatenate(xls, axis=0)

    shapes = dict(layers=L, D0=D0, HC=HC, NG16=NG16)
    nc = _build(plan, shapes)
    nc.m = get_hw_module(nc.m)

    # shared (replicated) inputs
    shared = {"xt": xt,
              "iotar": np.broadcast_to(np.arange(128, dtype=np.float32),
                                       (128, 128)).copy(),
              "iotac": np.arange(128, dtype=np.float32).reshape(128, 1)}
    for l in range(L):
        p = conv_params[l]
        dl = x.shape[1] if l == 0 else HC
        kc = max(1, dl // 128)
        for nm, pair in (("wqs", ("Wq", "Ws")), ("wkv", ("Wk", "Wv"))):
            Wc = np.concatenate([p[pair[0]], p[pair[1]]], axis=1)
            Wfull = np.zeros((kc * 128, 2 * HC), np.float32)
            Wfull[:dl] = Wc
            shared[f"{nm}{l}"] = np.ascontiguousarray(
                Wfull.reshape(kc, 128, 2 * HC).transpose(1, 0, 2).reshape(128, kc * 2 * HC))
        for bn in "qkvs":
            shared[f"b{bn}{l}"] = np.broadcast_to(p["b" + bn], (128, HC)).astype(np.float32).copy()
    for i in range(3):
        W, b = mlp_params[i]
        shared[f"wm{i}"] = W
        shared[f"bm{i}"] = b.reshape(-1, 1)
    for i in range(3):
        W, b = reg_params[i]
        shared[f"wr{i}"] = W
        shared[f"br{i}"] = b.reshape(-1, 1)

    in_maps = []
    for c in range(N_CORES):
        m = dict(shared)
        m["xl"] = xls[c]
        m["srcs"], m["dm"] = plan["metas"][c]
        m["gpool"] = gpools[c]
        m["gt"] = gts[c]
        in_maps.append(m)

    res = bass_utils.run_bass_kernel_spmd(nc, in_maps, core_ids=list(range(N_CORES)))
    out = np.concatenate([res.results[c]["y"][0] for c in range(N_CORES)])
    _LAST.update(nc=nc, in_maps=in_maps, plan=plan)
    return out.astype(np.float32)


# revision 9
# speedup vs baseline: 1.1047x; 1.1047x over previous
"""TransformerConv GNN (CircuitGNN) on 8 Trainium2 NeuronCores.

Strategy:
 - Shard graphs across 8 cores at graph boundaries (pooling stays local).
 - Per core, nodes are padded to P_N rows and grouped into 128-node blocks.
 - Edges are assigned to the core owning their dst, sorted by dst, grouped by
   dst block, split into 128-edge tiles.
 - Blocks are REORDERED per core (descending tile count) so that a single SPMD
   program with per-position tile counts T_pos = max over cores fits every
   core; shorter cores run masked pad tiles.
 - Per layer: node phase computes q/skip per block (dense matmuls, fp32r),
   edge tiles gather h[src] rows from a replicated full-h table via indirect
   DMA, compute k/v and attention on-chip, and scatter per-dst sums via
   one-hot matmuls into PSUM.  Softmax is computed without the segment-max
   shift (alpha is bounded, mathematically identical).
 - After layers 0..3 an AllGather replicates the new h to every core.
 - Layer 4 feeds a fused global-mean-pool (one-hot matmul), then the global
   MLP + regressor head run per core on its 64 graphs in transposed layout.
"""

import math
import numpy as np

N_CORES = 8
DK = 32  # channels per head (sqrt scaling)


# --------------------------------------------------------------------------
# host-side planning
# --------------------------------------------------------------------------

def _plan(x, edge_index, batch, n_graphs):
    N = x.shape[0]
    src_g = np.asarray(edge_index[0], np.int64)
    dst_g = np.asarray(edge_index[1], np.int64)
    gpc = n_graphs // N_CORES
    # node range per core (batch is sorted)
    gstart = np.searchsorted(batch, np.arange(n_graphs + 1))
    bounds = gstart[np.arange(0, n_graphs + 1, gpc)]          # [9]
    Nc = np.diff(bounds)
    P_N = max(128, int(math.ceil(Nc.max() / 128.0)) * 128)
    NB = P_N // 128

    owner = np.searchsorted(bounds, np.arange(N), side="right") - 1
    loc = np.arange(N) - bounds[owner]

    cores = []
    profiles = np.zeros((N_CORES, NB), np.int64)
    perms = []
    for c in range(N_CORES):
        em = owner[dst_g] == c
        es = src_g[em]
        ed = dst_g[em] - bounds[c]
        order = np.argsort(ed, kind="stable")
        es, ed = es[order], ed[order]
        blk = ed // 128
        ecnt = np.bincount(blk, minlength=NB)
        tcnt = -(-ecnt // 128)
        perm = np.argsort(-tcnt, kind="stable")               # pos -> orig blk
        perms.append(perm)
        profiles[c] = tcnt[perm]
        cores.append((es, ed, blk, ecnt, tcnt))

    # new node numbering: node in orig block b of core c sits at block
    # invperm[b] in the new order
    new_gid = np.empty(N, np.int64)
    invperms = []
    for c in range(N_CORES):
        invperm = np.empty(NB, np.int64)
        invperm[perms[c]] = np.arange(NB)
        invperms.append(invperm)
        m = owner == c
        lc = loc[m]
        new_gid[m] = c * P_N + invperm[lc // 128] * 128 + (lc % 128)

    T_pos = profiles.max(axis=0)                              # [NB]
    T_total = int(T_pos.sum())

    # per-core per-tile metadata
    metas = []
    for c in range(N_CORES):
        es, ed, blk, ecnt, tcnt = cores[c]
        estart = np.concatenate([[0], np.cumsum(ecnt)])
        srcs = np.zeros((T_total, 128), np.int32)
        dm = np.zeros((T_total, 128, 2), np.float32)
        t0 = 0
        for pos in range(NB):
            b = perms[c][pos]
            k = int(ecnt[b])
            base = int(estart[b])
            for t in range(int(T_pos[pos])):
                lo = t * 128
                n = max(0, min(128, k - lo))
                if n > 0:
                    sl = slice(base + lo, base + lo + n)
                    srcs[t0 + t, :n] = new_gid[es[sl]]
                    dm[t0 + t, :n, 0] = (ed[sl] % 128).astype(np.float32)
                    dm[t0 + t, :n, 1] = 1.0
            t0 += int(T_pos[pos])
        metas.append((srcs.reshape(-1, 1), dm.reshape(-1, 2)))

    return dict(P_N=P_N, NB=NB, bounds=bounds, Nc=Nc, owner=owner,
                perms=perms, invperms=invperms, T_pos=T_pos,
                T_total=T_total, metas=metas, gpc=gpc, gstart=gstart)


def _per_core_tables(plan, x_pad, batch, global_features, n_graphs):
    """xl [P_N, D0], gpool [NB*128, gpc], gT [n_glob, gpc] per core."""
    P_N, NB, gpc = plan["P_N"], plan["NB"], plan["gpc"]
    bounds, perms = plan["bounds"], plan["perms"]
    gstart = plan["gstart"]
    D0 = x_pad.shape[1]
    cnt = np.diff(gstart).astype(np.float32)
    xls, gpools, gts = [], [], []
    for c in range(N_CORES):
        n0, n1 = int(bounds[c]), int(bounds[c + 1])
        NcL = n1 - n0
        xl = np.zeros((P_N, D0), np.float32)
        gp = np.zeros((NB, 128, gpc), np.float32)
        for pos in range(NB):
            b = int(perms[c][pos])
            lo, hi = b * 128, min(b * 128 + 128, NcL)
            if lo < NcL:
                n = hi - lo
                xl[pos * 128:pos * 128 + n] = x_pad[n0 + lo:n0 + hi]
                gl = batch[n0 + lo:n0 + hi] - c * gpc
                w = 1.0 / np.maximum(cnt[c * gpc + gl], 1.0)
                gp[pos, np.arange(n), gl] = w
        xls.append(xl)
        gpools.append(gp.reshape(-1, gpc))
        gts.append(np.ascontiguousarray(
            global_features[c * gpc:(c + 1) * gpc].T.astype(np.float32)))
    return xls, gpools, gts


# --------------------------------------------------------------------------
# device program
# --------------------------------------------------------------------------

def _build(plan, shapes):
    import concourse.bacc as bacc
    import concourse.bass as bass
    import concourse.mybir as mybir
    import concourse.tile as tile
    from concourse.masks import make_identity

    f32 = mybir.dt.float32
    f32r = mybir.dt.float32r
    i32 = mybir.dt.int32
    AF = mybir.ActivationFunctionType
    OP = mybir.AluOpType

    P_N, NB, T_pos, T_total = plan["P_N"], plan["NB"], plan["T_pos"], plan["T_total"]
    gpc = plan["gpc"]
    L = shapes["layers"]            # number of conv layers
    D0 = shapes["D0"]               # padded input feature dim (128)
    HC = shapes["HC"]               # 256
    NG16 = shapes["NG16"]           # mlp hidden (16)
    REGIN = HC + NG16

    nc = bacc.Bacc("TRN2", target_bir_lowering=False, debug=False,
                   enable_asserts=False, num_devices=N_CORES,
                   num_swdge_queues=4)

    def din(layer):
        return D0 if layer == 0 else HC

    # ---- dram tensors -----------------------------------------------------
    bf16 = mybir.dt.bfloat16
    xt = nc.dram_tensor("xt", [N_CORES * P_N, D0], bf16, kind="ExternalInput").ap()
    xl = nc.dram_tensor("xl", [P_N, D0], f32, kind="ExternalInput").ap()
    meta = nc.dram_tensor("meta", [T_total * 128, 3], i32, kind="ExternalInput").ap()
    gpool = nc.dram_tensor("gpool", [NB * 128, gpc], f32, kind="ExternalInput").ap()
    gt_in = nc.dram_tensor("gt", [8, gpc], f32, kind="ExternalInput").ap()
    iotar = nc.dram_tensor("iotar", [128, 128], f32, kind="ExternalInput").ap()
    wqs, wkv, bqb, bkb, bvb, bsb = [], [], [], [], [], []
    for l in range(L):
        kc = din(l) // 128
        wqs.append(nc.dram_tensor(f"wqs{l}", [128, kc * 2 * HC], f32, kind="ExternalInput").ap())
        wkv.append(nc.dram_tensor(f"wkv{l}", [128, kc * 2 * HC], f32, kind="ExternalInput").ap())
        bqb.append(nc.dram_tensor(f"bq{l}", [128, HC], f32, kind="ExternalInput").ap())
        bkb.append(nc.dram_tensor(f"bk{l}", [128, HC], f32, kind="ExternalInput").ap())
        bvb.append(nc.dram_tensor(f"bv{l}", [128, HC], f32, kind="ExternalInput").ap())
        bsb.append(nc.dram_tensor(f"bs{l}", [128, HC], f32, kind="ExternalInput").ap())
    wm = []
    for i, shp in enumerate([[8, NG16], [NG16, NG16], [NG16, NG16]]):
        wm.append(nc.dram_tensor(f"wm{i}", shp, f32, kind="ExternalInput").ap())
    bm = [nc.dram_tensor(f"bm{i}", [NG16, 1], f32, kind="ExternalInput").ap()
          for i in range(3)]
    wr = []
    for i, shp in enumerate([[REGIN, NG16], [NG16, NG16], [NG16, 1]]):
        wr.append(nc.dram_tensor(f"wr{i}", shp, f32, kind="ExternalInput").ap())
    br = [nc.dram_tensor("br0", [NG16, 1], f32, kind="ExternalInput").ap(),
          nc.dram_tensor("br1", [NG16, 1], f32, kind="ExternalInput").ap(),
          nc.dram_tensor("br2", [1, 1], f32, kind="ExternalInput").ap()]
    y = nc.dram_tensor("y", [1, gpc], f32, kind="ExternalOutput").ap()

    hloc = [nc.dram_tensor(f"hloc{l}", [P_N, HC], f32) for l in range(1, L)]
    hlocb = [nc.dram_tensor(f"hlocb{l}", [P_N, HC], bf16) for l in range(1, L)]
    hfull = [nc.dram_tensor(f"hfull{l}", [N_CORES * P_N, HC], bf16,
                            addr_space="Shared") for l in range(1, L)]

    inv_sqrt_dk = 1.0 / math.sqrt(DK)

    with tile.TileContext(nc) as tc:
        with tc.tile_pool(name="const", bufs=1) as cp, \
             tc.tile_pool(name="wpool", bufs=2) as wp, \
             tc.tile_pool(name="nodep", bufs=3) as np_, \
             tc.tile_pool(name="edgep", bufs=6) as ep, \
             tc.tile_pool(name="psA", bufs=1, space="PSUM") as psA, \
             tc.tile_pool(name="psB", bufs=2, space="PSUM") as psB, \
             tc.tile_pool(name="psC", bufs=2, space="PSUM") as psC, \
             tc.tile_pool(name="headp", bufs=1) as hp:

            ident = cp.tile([128, 128], f32)
            make_identity(nc, ident[:])
            ident_r = cp.tile([128, 128], f32r)
            nc.vector.tensor_copy(ident_r[:], ident[:])
            ident_b = cp.tile([128, 128], bf16)
            nc.vector.tensor_copy(ident_b[:], ident[:])
            iotar_s = cp.tile([128, 128], f32)
            nc.sync.dma_start(iotar_s[:], iotar[:])

            pool_acc = hp.tile([gpc, HC], f32)
            nc.vector.memset(pool_acc[:], 0.0)

            for l in range(L):
                D = din(l)
                KC = D // 128
                htab = xt if l == 0 else hfull[l - 1][:]
                hl = xl if l == 0 else hloc[l - 1][:]

                wqs_s = wp.tile([128, KC * 2 * HC], f32r, tag="wqs")
                wkv_s = wp.tile([128, KC * 2 * HC], bf16, tag="wkv")
                wtmp = wp.tile([128, KC * 2 * HC], f32, tag="wtmp")
                nc.sync.dma_start(wtmp[:], wqs[l][:])
                nc.vector.tensor_copy(wqs_s[:], wtmp[:])
                wtmp2 = wp.tile([128, KC * 2 * HC], f32, tag="wtmp")
                nc.sync.dma_start(wtmp2[:], wkv[l][:])
                nc.vector.tensor_copy(wkv_s[:], wtmp2[:])
                bq_s = wp.tile([128, HC], f32, tag="bq")
                nc.sync.dma_start(bq_s[:], bqb[l][:])
                bk_s = wp.tile([128, HC], f32, tag="bk")
                nc.sync.dma_start(bk_s[:], bkb[l][:])
                bv_s = wp.tile([128, HC], f32, tag="bv")
                nc.sync.dma_start(bv_s[:], bvb[l][:])
                bs_s = wp.tile([128, HC], f32, tag="bs")
                nc.sync.dma_start(bs_s[:], bsb[l][:])

                gtile0 = 0
                for pos in range(NB):
                    # ---------------- node phase ----------------
                    hb = np_.tile([128, D], f32, tag="hb")
                    nc.sync.dma_start(hb[:], hl[pos * 128:(pos + 1) * 128, :])
                    hbT = np_.tile([128, KC * 128], f32r, tag="hbT")
                    for kc in range(KC):
                        tp = psB.tile([128, 128], f32, tag="tp", space="PSUM")
                        nc.tensor.transpose(tp[:], hb[:, kc * 128:(kc + 1) * 128], ident[:])
                        nc.scalar.copy(hbT[:, kc * 128:(kc + 1) * 128], tp[:])
                    qs_ps = psA.tile([128, 2 * HC], f32, tag="qs", space="PSUM")
                    for kc in range(KC):
                        nc.tensor.matmul(qs_ps[:], hbT[:, kc * 128:(kc + 1) * 128],
                                         wqs_s[:, kc * 2 * HC:(kc + 1) * 2 * HC],
                                         start=(kc == 0), stop=(kc == KC - 1))
                    q_s = np_.tile([128, HC], f32r, tag="q_s")
                    nc.vector.tensor_add(q_s[:], qs_ps[:, 0:HC], bq_s[:])

                    # ---------------- edge tiles ----------------
                    TP = int(T_pos[pos])
                    agg = None
                    if TP > 0:
                        agg = psA.tile([128, 8 + HC], f32, tag="agg", space="PSUM")
                    for t in range(TP):
                        gt_i = gtile0 + t
                        mi = ep.tile([128, 3], i32, tag="mi")
                        nc.sync.dma_start(mi[:], meta[gt_i * 128:(gt_i + 1) * 128, :])
                        mf = mi[:, 1:3].bitcast(f32)
                        gat = ep.tile([128, D], bf16, tag="gat")
                        gd = nc.gpsimd.indirect_dma_start(
                            out=gat[:], out_offset=None, in_=htab,
                            in_offset=bass.IndirectOffsetOnAxis(ap=mi[:, :1], axis=0))
                        qn = gt_i % 4
                        if qn:
                            gd.ins.queue = f"qPoolDynamic{qn}"
                        gT = ep.tile([128, KC * 128], bf16, tag="gT")
                        for kc in range(KC):
                            tp_b = psB.tile([128, 128], bf16, tag="tp", space="PSUM")
                            nc.tensor.transpose(tp_b[:], gat[:, kc * 128:(kc + 1) * 128],
                                                ident_b[:])
                            nc.scalar.copy(gT[:, kc * 128:(kc + 1) * 128], tp_b[:])
                        kv_ps = psB.tile([128, 2 * HC], f32, tag="kv", space="PSUM")
                        for kc in range(KC):
                            nc.tensor.matmul(kv_ps[:], gT[:, kc * 128:(kc + 1) * 128],
                                             wkv_s[:, kc * 2 * HC:(kc + 1) * 2 * HC],
                                             start=(kc == 0), stop=(kc == KC - 1))
                        ktile = ep.tile([128, HC], f32, tag="ktile")
                        nc.vector.tensor_add(ktile[:], kv_ps[:, 0:HC], bk_s[:])
                        # one-hot matrices from dst_rel
                        S = ep.tile([128, 128], f32r, tag="S")
                        nc.vector.tensor_tensor(
                            out=S[:], in0=mf[:, 0:1].to_broadcast([128, 128]),
                            in1=iotar_s[:], op=OP.is_equal)
                        tpr = psB.tile([128, 128], f32r, tag="tp", space="PSUM")
                        nc.tensor.transpose(tpr[:], S[:], ident_r[:])
                        S_T = ep.tile([128, 128], f32r, tag="S_T")
                        nc.scalar.copy(S_T[:], tpr[:])
                        qdst = psC.tile([128, HC], f32, tag="qdst", space="PSUM")
                        nc.tensor.matmul(qdst[:], S_T[:], q_s[:], start=True, stop=True)
                        prod = ep.tile([128, HC], f32, tag="prod")
                        nc.vector.tensor_tensor(out=prod[:], in0=qdst[:], in1=ktile[:],
                                                op=OP.mult)
                        alpha = ep.tile([128, 8], f32, tag="alpha")
                        nc.vector.tensor_reduce(
                            out=alpha[:], in_=prod[:].rearrange("p (h c) -> p h c", c=DK),
                            axis=mybir.AxisListType.X, op=OP.add)
                        et = ep.tile([128, 8], f32, tag="et")
                        nc.scalar.activation(et[:], alpha[:], AF.Exp,
                                             scale=inv_sqrt_dk)
                        rhs_t = ep.tile([128, 8 + HC], f32r, tag="rhs_t")
                        nc.vector.tensor_tensor(
                            out=rhs_t[:, 0:8], in0=et[:],
                            in1=mf[:, 1:2].to_broadcast([128, 8]), op=OP.mult)
                        nc.vector.tensor_tensor(
                            out=rhs_t[:, 8:8 + HC].rearrange("p (h c) -> p h c", c=DK),
                            in0=rhs_t[:, 0:8].to_broadcast([128, 8, DK]),
                            in1=kv_ps[:, HC:2 * HC].rearrange("p (h c) -> p h c", c=DK),
                            op=OP.mult)
                        nc.tensor.matmul(agg[:], S[:], rhs_t[:],
                                         start=(t == 0), stop=(t == TP - 1))
                    gtile0 += TP

                    # ---------------- epilogue ----------------
                    hout_dt = f32r if l == L - 1 else f32
                    hout = np_.tile([128, HC], hout_dt, tag="hout")
                    if TP > 0:
                        d1 = np_.tile([128, 8], f32, tag="d1")
                        nc.vector.tensor_scalar_add(d1[:], agg[:, 0:8], 1e-16)
                        rec = np_.tile([128, 8], f32, tag="rec")
                        nc.vector.reciprocal(rec[:], d1[:])
                        bvd = np_.tile([128, HC], f32, tag="bvd")
                        nc.vector.tensor_tensor(
                            out=bvd[:].rearrange("p (h c) -> p h c", c=DK),
                            in0=bv_s[:].rearrange("p (h c) -> p h c", c=DK),
                            in1=agg[:, 0:8].to_broadcast([128, 8, DK]), op=OP.mult)
                        p2 = np_.tile([128, HC], f32, tag="p2")
                        nc.vector.tensor_add(p2[:], agg[:, 8:8 + HC], bvd[:])
                        u = np_.tile([128, HC], f32, tag="u")
                        nc.vector.tensor_tensor(
                            out=u[:].rearrange("p (h c) -> p h c", c=DK),
                            in0=p2[:].rearrange("p (h c) -> p h c", c=DK),
                            in1=rec[:].to_broadcast([128, 8, DK]), op=OP.mult)
                        w1 = np_.tile([128, HC], f32, tag="w1")
                        nc.vector.tensor_add(w1[:], u[:], qs_ps[:, HC:2 * HC])
                        w2 = np_.tile([128, HC], f32, tag="w2")
                        nc.vector.tensor_add(w2[:], w1[:], bs_s[:])
                    else:
                        w2 = np_.tile([128, HC], f32, tag="w2")
                        nc.vector.tensor_add(w2[:], qs_ps[:, HC:2 * HC], bs_s[:])
                    nc.scalar.activation(hout[:], w2[:], AF.Relu)

                    if l < L - 1:
                        nc.sync.dma_start(hloc[l][pos * 128:(pos + 1) * 128, :], hout[:])
                        houtb = np_.tile([128, HC], bf16, tag="houtb")
                        nc.vector.tensor_copy(houtb[:], hout[:])
                        nc.sync.dma_start(hlocb[l][pos * 128:(pos + 1) * 128, :], houtb[:])
                    else:
                        gps = np_.tile([128, gpc], f32, tag="gps")
                        nc.sync.dma_start(gps[:], gpool[pos * 128:(pos + 1) * 128, :])
                        gpr = np_.tile([128, gpc], f32r, tag="gpr")
                        nc.vector.tensor_copy(gpr[:], gps[:])
                        pl_ps = psC.tile([gpc, HC], f32, tag="qdst", space="PSUM")
                        nc.tensor.matmul(pl_ps[:], gpr[:], hout[:], start=True, stop=True)
                        nc.vector.tensor_add(pool_acc[:], pool_acc[:], pl_ps[:])

                if l < L - 1:
                    nc.gpsimd.collective_compute(
                        "AllGather", mybir.AluOpType.bypass,
                        replica_groups=[list(range(N_CORES))],
                        ins=[hlocb[l][:].opt()],
                        outs=[hfull[l][:].opt()],
                    )

            # ------------- head: pool -> concat -> regressor ----------------
            pool_r = hp.tile([gpc, HC], f32r)
            nc.vector.tensor_copy(pool_r[:], pool_acc[:])
            poolT = hp.tile([128, 2 * gpc], f32r)
            for kc in range(2):
                tp = psB.tile([128, 128], f32r, tag="tp", space="PSUM")
                nc.tensor.transpose(tp[:, 0:gpc],
                                    pool_r[:, kc * 128:(kc + 1) * 128],
                                    ident_r[0:gpc, 0:gpc])
                nc.scalar.copy(poolT[:, kc * gpc:(kc + 1) * gpc], tp[:, 0:gpc])

            gt_s = hp.tile([8, gpc], f32r)
            gtmp = hp.tile([8, gpc], f32)
            nc.sync.dma_start(gtmp[:], gt_in[:])
            nc.vector.tensor_copy(gt_s[:], gtmp[:])

            def load_small(ap_in, p, q_, tagn):
                tt = hp.tile([p, q_], f32, tag=tagn + "f")
                nc.sync.dma_start(tt[:], ap_in)
                rr = hp.tile([p, q_], f32r, tag=tagn)
                nc.vector.tensor_copy(rr[:], tt[:])
                return rr

            cur = gt_s
            for i in range(3):
                wmi = load_small(wm[i][:], [8, NG16, NG16][i], NG16, f"wm{i}")
                bmi = hp.tile([NG16, 1], f32, tag=f"bm{i}")
                nc.sync.dma_start(bmi[:], bm[i][:])
                zz = psC.tile([NG16, gpc], f32, tag="qdst", space="PSUM")
                nc.tensor.matmul(zz[:], wmi[:], cur[:], start=True, stop=True)
                nxt = hp.tile([NG16, gpc], f32r, tag=f"m{i}")
                nc.scalar.activation(nxt[:], zz[:], AF.Relu, bias=bmi[:, 0:1])
                cur = nxt

            wr0a = load_small(wr[0][0:128, :], 128, NG16, "wr0a")
            wr0b = load_small(wr[0][128:256, :], 128, NG16, "wr0b")
            wr0c = load_small(wr[0][256:REGIN, :], NG16, NG16, "wr0c")
            z1 = psC.tile([NG16, gpc], f32, tag="qdst", space="PSUM")
            nc.tensor.matmul(z1[:], wr0a[:], poolT[:, 0:gpc], start=True, stop=False)
            nc.tensor.matmul(z1[:], wr0b[:], poolT[:, gpc:2 * gpc], start=False, stop=False)
            nc.tensor.matmul(z1[:], wr0c[:], cur[:], start=False, stop=True)
            br0_s = hp.tile([NG16, 1], f32)
            nc.sync.dma_start(br0_s[:], br[0][:])
            z1s = hp.tile([NG16, gpc], f32r)
            nc.scalar.activation(z1s[:], z1[:], AF.Relu, bias=br0_s[:, 0:1])

            wr1s = load_small(wr[1][:], NG16, NG16, "wr1")
            z2 = psC.tile([NG16, gpc], f32, tag="qdst", space="PSUM")
            nc.tensor.matmul(z2[:], wr1s[:], z1s[:], start=True, stop=True)
            br1_s = hp.tile([NG16, 1], f32)
            nc.sync.dma_start(br1_s[:], br[1][:])
            z2s = hp.tile([NG16, gpc], f32r)
            nc.scalar.activation(z2s[:], z2[:], AF.Relu, bias=br1_s[:, 0:1])

            wr2s = load_small(wr[2][:], NG16, 1, "wr2")
            z3 = psC.tile([1, gpc], f32, tag="qdst", space="PSUM")
            nc.tensor.matmul(z3[:], wr2s[:], z2s[:], start=True, stop=True)
            br2_s = hp.tile([1, 1], f32)
            nc.sync.dma_start(br2_s[:], br[2][:])
            ys = hp.tile([1, gpc], f32)
            nc.scalar.activation(ys[:], z3[:], AF.Identity, bias=br2_s[:, 0:1])
            nc.sync.dma_start(y[:], ys[:])

    nc.compile()
    return nc


# --------------------------------------------------------------------------
# entry point
# --------------------------------------------------------------------------

_LAST = {}


def kernel(x, edge_index, batch, global_features, conv_params, mlp_params,
           reg_params):
    from concourse import bass_utils
    from concourse.bass_interp import get_hw_module

    x = np.asarray(x, np.float32)
    edge_index = np.asarray(edge_index)
    batch_np = np.asarray(batch, np.int64)
    global_features = np.asarray(global_features, np.float32)
    conv_params = [{k: np.asarray(v, np.float32) for k, v in p.items()}
                   for p in conv_params]
    mlp_params = [(np.asarray(W, np.float32), np.asarray(b, np.float32))
                  for W, b in mlp_params]
    reg_params = [(np.asarray(W, np.float32), np.asarray(b, np.float32))
                  for W, b in reg_params]

    n_graphs = global_features.shape[0]
    HC = conv_params[0]["Wq"].shape[1]
    L = len(conv_params)
    D0 = 128
    NG16 = mlp_params[0][0].shape[1]
    gpc = n_graphs // N_CORES

    plan = _plan(x, edge_index, batch_np, n_graphs)
    P_N, NB, T_total = plan["P_N"], plan["NB"], plan["T_total"]

    x_pad = np.zeros((x.shape[0], D0), np.float32)
    x_pad[:, :x.shape[1]] = x
    xls, gpools, gts = _per_core_tables(plan, x_pad, batch_np, global_features,
                                        n_graphs)
    xt = np.concatenate(xls, axis=0)

    shapes = dict(layers=L, D0=D0, HC=HC, NG16=NG16)
    nc = _build(plan, shapes)
    nc.m = get_hw_module(nc.m)

    import ml_dtypes
    # shared (replicated) inputs
    shared = {"xt": xt.astype(ml_dtypes.bfloat16),
              "iotar": np.broadcast_to(np.arange(128, dtype=np.float32),
                                       (128, 128)).copy()}
    for l in range(L):
        p = conv_params[l]
        dl = x.shape[1] if l == 0 else HC
        kc = max(1, dl // 128)
        for nm, pair in (("wqs", ("Wq", "Ws")), ("wkv", ("Wk", "Wv"))):
            Wc = np.concatenate([p[pair[0]], p[pair[1]]], axis=1)
            Wfull = np.zeros((kc * 128, 2 * HC), np.float32)
            Wfull[:dl] = Wc
            shared[f"{nm}{l}"] = np.ascontiguousarray(
                Wfull.reshape(kc, 128, 2 * HC).transpose(1, 0, 2).reshape(128, kc * 2 * HC))
        for bn in "qkvs":
            shared[f"b{bn}{l}"] = np.broadcast_to(p["b" + bn], (128, HC)).astype(np.float32).copy()
    for i in range(3):
        W, b = mlp_params[i]
        shared[f"wm{i}"] = W
        shared[f"bm{i}"] = b.reshape(-1, 1)
    for i in range(3):
        W, b = reg_params[i]
        shared[f"wr{i}"] = W
        shared[f"br{i}"] = b.reshape(-1, 1)

    in_maps = []
    for c in range(N_CORES):
        m = dict(shared)
        m["xl"] = xls[c]
        srcs_c, dm_c = plan["metas"][c]
        mcomb = np.empty((srcs_c.shape[0], 3), np.int32)
        mcomb[:, 0] = srcs_c[:, 0]
        mcomb[:, 1:3] = dm_c.view(np.int32)
        m["meta"] = mcomb
        m["gpool"] = gpools[c]
        m["gt"] = gts[c]
        in_maps.append(m)

    res = bass_utils.run_bass_kernel_spmd(nc, in_maps, core_ids=list(range(N_CORES)))
    out = np.concatenate([res.results[c]["y"][0] for c in range(N_CORES)])
    _LAST.update(nc=nc, in_maps=in_maps, plan=plan)
    return out.astype(np.float32)


# revision 12
# speedup vs baseline: 1.1441x; 1.0357x over previous
"""TransformerConv GNN (CircuitGNN) on 8 Trainium2 NeuronCores.

Strategy:
 - Shard graphs across 8 cores at graph boundaries (pooling stays local).
 - Per core, nodes are padded to P_N rows and grouped into 128-node blocks.
 - Edges are assigned to the core owning their dst, sorted by dst, grouped by
   dst block, split into 128-edge tiles.
 - Blocks are REORDERED per core (descending tile count) so that a single SPMD
   program with per-position tile counts T_pos = max over cores fits every
   core; shorter cores run masked pad tiles.
 - Per layer: node phase computes q/skip per block (dense matmuls, fp32r),
   edge tiles gather h[src] rows from a replicated full-h table via indirect
   DMA, compute k/v and attention on-chip, and scatter per-dst sums via
   one-hot matmuls into PSUM.  Softmax is computed without the segment-max
   shift (alpha is bounded, mathematically identical).
 - After layers 0..3 an AllGather replicates the new h to every core.
 - Layer 4 feeds a fused global-mean-pool (one-hot matmul), then the global
   MLP + regressor head run per core on its 64 graphs in transposed layout.
"""

import math
import numpy as np

N_CORES = 8
DK = 32  # channels per head (sqrt scaling)


# --------------------------------------------------------------------------
# host-side planning
# --------------------------------------------------------------------------

def _plan(x, edge_index, batch, n_graphs):
    N = x.shape[0]
    src_g = np.asarray(edge_index[0], np.int64)
    dst_g = np.asarray(edge_index[1], np.int64)
    gpc = n_graphs // N_CORES
    # node range per core (batch is sorted)
    gstart = np.searchsorted(batch, np.arange(n_graphs + 1))
    bounds = gstart[np.arange(0, n_graphs + 1, gpc)]          # [9]
    Nc = np.diff(bounds)
    P_N = max(128, int(math.ceil(Nc.max() / 128.0)) * 128)
    NB = P_N // 128

    owner = np.searchsorted(bounds, np.arange(N), side="right") - 1
    loc = np.arange(N) - bounds[owner]

    cores = []
    profiles = np.zeros((N_CORES, NB), np.int64)
    perms = []
    for c in range(N_CORES):
        em = owner[dst_g] == c
        es = src_g[em]
        ed = dst_g[em] - bounds[c]
        order = np.argsort(ed, kind="stable")
        es, ed = es[order], ed[order]
        blk = ed // 128
        ecnt = np.bincount(blk, minlength=NB)
        tcnt = -(-ecnt // 128)
        perm = np.argsort(-tcnt, kind="stable")               # pos -> orig blk
        perms.append(perm)
        profiles[c] = tcnt[perm]
        cores.append((es, ed, blk, ecnt, tcnt))

    # new node numbering: node in orig block b of core c sits at block
    # invperm[b] in the new order
    new_gid = np.empty(N, np.int64)
    invperms = []
    for c in range(N_CORES):
        invperm = np.empty(NB, np.int64)
        invperm[perms[c]] = np.arange(NB)
        invperms.append(invperm)
        m = owner == c
        lc = loc[m]
        new_gid[m] = c * P_N + invperm[lc // 128] * 128 + (lc % 128)

    T_pos = profiles.max(axis=0)                              # [NB]
    T_total = int(T_pos.sum())

    # per-core per-tile metadata
    metas = []
    for c in range(N_CORES):
        es, ed, blk, ecnt, tcnt = cores[c]
        estart = np.concatenate([[0], np.cumsum(ecnt)])
        srcs = np.zeros((T_total, 128), np.int32)
        dm = np.zeros((T_total, 128, 2), np.float32)
        t0 = 0
        for pos in range(NB):
            b = perms[c][pos]
            k = int(ecnt[b])
            base = int(estart[b])
            for t in range(int(T_pos[pos])):
                lo = t * 128
                n = max(0, min(128, k - lo))
                if n > 0:
                    sl = slice(base + lo, base + lo + n)
                    srcs[t0 + t, :n] = new_gid[es[sl]]
                    dm[t0 + t, :n, 0] = (ed[sl] % 128).astype(np.float32)
                    dm[t0 + t, :n, 1] = 1.0
            t0 += int(T_pos[pos])
        metas.append((srcs.reshape(-1, 1), dm.reshape(-1, 2)))

    return dict(P_N=P_N, NB=NB, bounds=bounds, Nc=Nc, owner=owner,
                perms=perms, invperms=invperms, T_pos=T_pos,
                T_total=T_total, metas=metas, gpc=gpc, gstart=gstart)


def _per_core_tables(plan, x_pad, batch, global_features, n_graphs):
    """xl [P_N, D0], gpool [NB*128, gpc], gT [n_glob, gpc] per core."""
    P_N, NB, gpc = plan["P_N"], plan["NB"], plan["gpc"]
    bounds, perms = plan["bounds"], plan["perms"]
    gstart = plan["gstart"]
    D0 = x_pad.shape[1]
    cnt = np.diff(gstart).astype(np.float32)
    xls, gpools, gts = [], [], []
    for c in range(N_CORES):
        n0, n1 = int(bounds[c]), int(bounds[c + 1])
        NcL = n1 - n0
        xl = np.zeros((P_N, D0), np.float32)
        gp = np.zeros((NB, 128, gpc), np.float32)
        for pos in range(NB):
            b = int(perms[c][pos])
            lo, hi = b * 128, min(b * 128 + 128, NcL)
            if lo < NcL:
                n = hi - lo
                xl[pos * 128:pos * 128 + n] = x_pad[n0 + lo:n0 + hi]
                gl = batch[n0 + lo:n0 + hi] - c * gpc
                w = 1.0 / np.maximum(cnt[c * gpc + gl], 1.0)
                gp[pos, np.arange(n), gl] = w
        xls.append(xl)
        gpools.append(gp.reshape(-1, gpc))
        gts.append(np.ascontiguousarray(
            global_features[c * gpc:(c + 1) * gpc].T.astype(np.float32)))
    return xls, gpools, gts


# --------------------------------------------------------------------------
# device program
# --------------------------------------------------------------------------

def _build(plan, shapes):
    import concourse.bacc as bacc
    import concourse.bass as bass
    import concourse.mybir as mybir
    import concourse.tile as tile
    from concourse.masks import make_identity

    f32 = mybir.dt.float32
    f32r = mybir.dt.float32r
    i32 = mybir.dt.int32
    AF = mybir.ActivationFunctionType
    OP = mybir.AluOpType

    P_N, NB, T_pos, T_total = plan["P_N"], plan["NB"], plan["T_pos"], plan["T_total"]
    gpc = plan["gpc"]
    L = shapes["layers"]            # number of conv layers
    D0 = shapes["D0"]               # padded input feature dim (128)
    HC = shapes["HC"]               # 256
    NG16 = shapes["NG16"]           # mlp hidden (16)
    REGIN = HC + NG16

    nc = bacc.Bacc("TRN2", target_bir_lowering=False, debug=False,
                   enable_asserts=False, num_devices=N_CORES,
                   num_swdge_queues=4)

    def din(layer):
        return D0 if layer == 0 else HC

    # ---- dram tensors -----------------------------------------------------
    bf16 = mybir.dt.bfloat16
    xt = nc.dram_tensor("xt", [N_CORES * P_N, D0], bf16, kind="ExternalInput").ap()
    xl = nc.dram_tensor("xl", [P_N, D0], f32, kind="ExternalInput").ap()
    meta = nc.dram_tensor("meta", [T_total * 128, 1], i32, kind="ExternalInput").ap()
    smat = nc.dram_tensor("smat", [T_total * 128, 128], bf16, kind="ExternalInput").ap()
    gpool = nc.dram_tensor("gpool", [NB * 128, gpc], f32, kind="ExternalInput").ap()
    gt_in = nc.dram_tensor("gt", [8, gpc], f32, kind="ExternalInput").ap()
    wqs, wkv, bqb, bkb, bvb, bsb = [], [], [], [], [], []
    for l in range(L):
        kc = din(l) // 128
        wqs.append(nc.dram_tensor(f"wqs{l}", [128, kc * 2 * HC], f32, kind="ExternalInput").ap())
        wkv.append(nc.dram_tensor(f"wkv{l}", [128, kc * 2 * HC], f32, kind="ExternalInput").ap())
        bqb.append(nc.dram_tensor(f"bq{l}", [128, HC], f32, kind="ExternalInput").ap())
        bkb.append(nc.dram_tensor(f"bk{l}", [128, HC], f32, kind="ExternalInput").ap())
        bvb.append(nc.dram_tensor(f"bv{l}", [128, HC], f32, kind="ExternalInput").ap())
        bsb.append(nc.dram_tensor(f"bs{l}", [128, HC], f32, kind="ExternalInput").ap())
    wm = []
    for i, shp in enumerate([[8, NG16], [NG16, NG16], [NG16, NG16]]):
        wm.append(nc.dram_tensor(f"wm{i}", shp, f32, kind="ExternalInput").ap())
    bm = [nc.dram_tensor(f"bm{i}", [NG16, 1], f32, kind="ExternalInput").ap()
          for i in range(3)]
    wr = []
    for i, shp in enumerate([[REGIN, NG16], [NG16, NG16], [NG16, 1]]):
        wr.append(nc.dram_tensor(f"wr{i}", shp, f32, kind="ExternalInput").ap())
    br = [nc.dram_tensor("br0", [NG16, 1], f32, kind="ExternalInput").ap(),
          nc.dram_tensor("br1", [NG16, 1], f32, kind="ExternalInput").ap(),
          nc.dram_tensor("br2", [1, 1], f32, kind="ExternalInput").ap()]
    y = nc.dram_tensor("y", [1, gpc], f32, kind="ExternalOutput").ap()

    hloc = [nc.dram_tensor(f"hloc{l}", [P_N, HC], f32) for l in range(1, L)]
    hlocb = [nc.dram_tensor(f"hlocb{l}", [P_N, HC], bf16) for l in range(1, L)]
    hfull = [nc.dram_tensor(f"hfull{l}", [N_CORES * P_N, HC], bf16,
                            addr_space="Shared") for l in range(1, L)]

    inv_sqrt_dk = 1.0 / math.sqrt(DK)

    with tile.TileContext(nc) as tc:
        with tc.tile_pool(name="const", bufs=1) as cp, \
             tc.tile_pool(name="wpool", bufs=2) as wp, \
             tc.tile_pool(name="nodep", bufs=3) as np_, \
             tc.tile_pool(name="edgep", bufs=6) as ep, \
             tc.tile_pool(name="psA", bufs=1, space="PSUM") as psA, \
             tc.tile_pool(name="psB", bufs=2, space="PSUM") as psB, \
             tc.tile_pool(name="psC", bufs=2, space="PSUM") as psC, \
             tc.tile_pool(name="headp", bufs=1) as hp:

            ident = cp.tile([128, 128], f32)
            make_identity(nc, ident[:])
            ident_r = cp.tile([128, 128], f32r)
            nc.vector.tensor_copy(ident_r[:], ident[:])
            ident_b = cp.tile([128, 128], bf16)
            nc.vector.tensor_copy(ident_b[:], ident[:])

            pool_acc = hp.tile([gpc, HC], f32)
            nc.vector.memset(pool_acc[:], 0.0)

            for l in range(L):
                D = din(l)
                KC = D // 128
                htab = xt if l == 0 else hfull[l - 1][:]
                hl = xl if l == 0 else hloc[l - 1][:]

                wqs_s = wp.tile([128, KC * 2 * HC], f32r, tag="wqs")
                wkv_s = wp.tile([128, KC * 2 * HC], bf16, tag="wkv")
                wtmp = wp.tile([128, KC * 2 * HC], f32, tag="wtmp")
                nc.sync.dma_start(wtmp[:], wqs[l][:])
                nc.vector.tensor_copy(wqs_s[:], wtmp[:])
                wtmp2 = wp.tile([128, KC * 2 * HC], f32, tag="wtmp")
                nc.sync.dma_start(wtmp2[:], wkv[l][:])
                nc.vector.tensor_copy(wkv_s[:], wtmp2[:])
                bq_s = wp.tile([128, HC], f32, tag="bq")
                nc.sync.dma_start(bq_s[:], bqb[l][:])
                bk_s = wp.tile([128, HC], f32, tag="bk")
                nc.sync.dma_start(bk_s[:], bkb[l][:])
                bv_s = wp.tile([128, HC], f32, tag="bv")
                nc.sync.dma_start(bv_s[:], bvb[l][:])
                bs_s = wp.tile([128, HC], f32, tag="bs")
                nc.sync.dma_start(bs_s[:], bsb[l][:])

                gtile0 = 0
                for pos in range(NB):
                    # ---------------- node phase ----------------
                    hb = np_.tile([128, D], f32, tag="hb")
                    nc.sync.dma_start(hb[:], hl[pos * 128:(pos + 1) * 128, :])
                    hbT = np_.tile([128, KC * 128], f32r, tag="hbT")
                    for kc in range(KC):
                        tp = psB.tile([128, 128], f32, tag="tp", space="PSUM")
                        nc.tensor.transpose(tp[:], hb[:, kc * 128:(kc + 1) * 128], ident[:])
                        nc.scalar.copy(hbT[:, kc * 128:(kc + 1) * 128], tp[:])
                    qs_ps = psA.tile([128, 2 * HC], f32, tag="qs", space="PSUM")
                    for kc in range(KC):
                        nc.tensor.matmul(qs_ps[:], hbT[:, kc * 128:(kc + 1) * 128],
                                         wqs_s[:, kc * 2 * HC:(kc + 1) * 2 * HC],
                                         start=(kc == 0), stop=(kc == KC - 1))
                    q_s = np_.tile([128, HC + 8], f32r, tag="q_s")
                    nc.vector.tensor_add(q_s[:, 0:HC], qs_ps[:, 0:HC], bq_s[:])
                    bkp = np_.tile([128, HC], f32, tag="bkp")
                    nc.vector.tensor_tensor(out=bkp[:], in0=q_s[:, 0:HC],
                                            in1=bk_s[:], op=OP.mult)
                    with nc.allow_low_precision(reason="f32r beta reduce"):
                        nc.vector.tensor_reduce(
                            out=q_s[:, HC:HC + 8],
                            in_=bkp[:].rearrange("p (h c) -> p h c", c=DK),
                            axis=mybir.AxisListType.X, op=OP.add)

                    # ---------------- edge tiles ----------------
                    TP = int(T_pos[pos])
                    agg = None
                    if TP > 0:
                        agg = psA.tile([128, 8 + HC], f32, tag="agg", space="PSUM")
                    for t in range(TP):
                        gt_i = gtile0 + t
                        mi = ep.tile([128, 1], i32, tag="mi")
                        nc.sync.dma_start(mi[:], meta[gt_i * 128:(gt_i + 1) * 128, :])
                        S_s = ep.tile([128, 128], bf16, tag="S_s")
                        nc.sync.dma_start(S_s[:], smat[gt_i * 128:(gt_i + 1) * 128, :])
                        gat = ep.tile([128, D], bf16, tag="gat")
                        gd = nc.gpsimd.indirect_dma_start(
                            out=gat[:], out_offset=None, in_=htab,
                            in_offset=bass.IndirectOffsetOnAxis(ap=mi[:, :1], axis=0))
                        qn = gt_i % 4
                        if qn:
                            gd.ins.queue = f"qPoolDynamic{qn}"
                        gT = ep.tile([128, KC * 128], bf16, tag="gT")
                        for kc in range(KC):
                            tp_b = psB.tile([128, 128], bf16, tag="tp", space="PSUM")
                            nc.tensor.transpose(tp_b[:], gat[:, kc * 128:(kc + 1) * 128],
                                                ident_b[:])
                            nc.scalar.copy(gT[:, kc * 128:(kc + 1) * 128], tp_b[:])
                        kv_ps = psB.tile([128, 2 * HC], f32, tag="kv", space="PSUM")
                        for kc in range(KC):
                            nc.tensor.matmul(kv_ps[:], gT[:, kc * 128:(kc + 1) * 128],
                                             wkv_s[:, kc * 2 * HC:(kc + 1) * 2 * HC],
                                             start=(kc == 0), stop=(kc == KC - 1))
                        ktile = ep.tile([128, HC], f32, tag="ktile")
                        nc.scalar.copy(ktile[:], kv_ps[:, 0:HC])
                        tpb = psB.tile([128, 128], bf16, tag="tp", space="PSUM")
                        nc.tensor.transpose(tpb[:], S_s[:], ident_b[:])
                        S_T = ep.tile([128, 128], f32r, tag="S_T")
                        nc.scalar.copy(S_T[:], tpb[:])
                        qdst = psC.tile([128, HC + 8], f32, tag="qdst", space="PSUM")
                        nc.tensor.matmul(qdst[:], S_T[:], q_s[:], start=True, stop=True)
                        prod = ep.tile([128, HC], f32, tag="prod")
                        nc.vector.tensor_tensor(out=prod[:], in0=qdst[:, 0:HC],
                                                in1=ktile[:], op=OP.mult)
                        alpha = ep.tile([128, 8], f32, tag="alpha")
                        nc.vector.tensor_reduce(
                            out=alpha[:], in_=prod[:].rearrange("p (h c) -> p h c", c=DK),
                            axis=mybir.AxisListType.X, op=OP.add)
                        alpha2 = ep.tile([128, 8], f32, tag="alpha2")
                        nc.vector.tensor_add(alpha2[:], alpha[:], qdst[:, HC:HC + 8])
                        rhs_t = ep.tile([128, 8 + HC], bf16, tag="rhs_t")
                        nc.scalar.activation(rhs_t[:, 0:8], alpha2[:], AF.Exp,
                                             scale=inv_sqrt_dk)
                        nc.vector.tensor_tensor(
                            out=rhs_t[:, 8:8 + HC].rearrange("p (h c) -> p h c", c=DK),
                            in0=rhs_t[:, 0:8].to_broadcast([128, 8, DK]),
                            in1=kv_ps[:, HC:2 * HC].rearrange("p (h c) -> p h c", c=DK),
                            op=OP.mult)
                        nc.tensor.matmul(agg[:], S_s[:], rhs_t[:],
                                         start=(t == 0), stop=(t == TP - 1))
                    gtile0 += TP

                    # ---------------- epilogue ----------------
                    hout_dt = f32r if l == L - 1 else f32
                    hout = np_.tile([128, HC], hout_dt, tag="hout")
                    if TP > 0:
                        d1 = np_.tile([128, 8], f32, tag="d1")
                        nc.vector.tensor_scalar_add(d1[:], agg[:, 0:8], 1e-16)
                        rec = np_.tile([128, 8], f32, tag="rec")
                        nc.vector.reciprocal(rec[:], d1[:])
                        bvd = np_.tile([128, HC], f32, tag="bvd")
                        nc.vector.tensor_tensor(
                            out=bvd[:].rearrange("p (h c) -> p h c", c=DK),
                            in0=bv_s[:].rearrange("p (h c) -> p h c", c=DK),
                            in1=agg[:, 0:8].to_broadcast([128, 8, DK]), op=OP.mult)
                        p2 = np_.tile([128, HC], f32, tag="p2")
                        nc.vector.tensor_add(p2[:], agg[:, 8:8 + HC], bvd[:])
                        u = np_.tile([128, HC], f32, tag="u")
                        nc.vector.tensor_tensor(
                            out=u[:].rearrange("p (h c) -> p h c", c=DK),
                            in0=p2[:].rearrange("p (h c) -> p h c", c=DK),
                            in1=rec[:].to_broadcast([128, 8, DK]), op=OP.mult)
                        w1 = np_.tile([128, HC], f32, tag="w1")
                        nc.vector.tensor_add(w1[:], u[:], qs_ps[:, HC:2 * HC])
                        w2 = np_.tile([128, HC], f32, tag="w2")
                        nc.vector.tensor_add(w2[:], w1[:], bs_s[:])
                    else:
                        w2 = np_.tile([128, HC], f32, tag="w2")
                        nc.vector.tensor_add(w2[:], qs_ps[:, HC:2 * HC], bs_s[:])
                    nc.scalar.activation(hout[:], w2[:], AF.Relu)

                    if l < L - 1:
                        nc.sync.dma_start(hloc[l][pos * 128:(pos + 1) * 128, :], hout[:])
                        houtb = np_.tile([128, HC], bf16, tag="houtb")
                        nc.vector.tensor_copy(houtb[:], hout[:])
                        nc.sync.dma_start(hlocb[l][pos * 128:(pos + 1) * 128, :], houtb[:])
                    else:
                        gps = np_.tile([128, gpc], f32, tag="gps")
                        nc.sync.dma_start(gps[:], gpool[pos * 128:(pos + 1) * 128, :])
                        gpr = np_.tile([128, gpc], f32r, tag="gpr")
                        nc.vector.tensor_copy(gpr[:], gps[:])
                        pl_ps = psC.tile([gpc, HC], f32, tag="qdst", space="PSUM")
                        nc.tensor.matmul(pl_ps[:], gpr[:], hout[:], start=True, stop=True)
                        nc.vector.tensor_add(pool_acc[:], pool_acc[:], pl_ps[:])

                if l < L - 1:
                    nc.gpsimd.collective_compute(
                        "AllGather", mybir.AluOpType.bypass,
                        replica_groups=[list(range(N_CORES))],
                        ins=[hlocb[l][:].opt()],
                        outs=[hfull[l][:].opt()],
                    )

            # ------------- head: pool -> concat -> regressor ----------------
            pool_r = hp.tile([gpc, HC], f32r)
            nc.vector.tensor_copy(pool_r[:], pool_acc[:])
            poolT = hp.tile([128, 2 * gpc], f32r)
            for kc in range(2):
                tp = psB.tile([128, 128], f32r, tag="tp", space="PSUM")
                nc.tensor.transpose(tp[:, 0:gpc],
                                    pool_r[:, kc * 128:(kc + 1) * 128],
                                    ident_r[0:gpc, 0:gpc])
                nc.scalar.copy(poolT[:, kc * gpc:(kc + 1) * gpc], tp[:, 0:gpc])

            gt_s = hp.tile([8, gpc], f32r)
            gtmp = hp.tile([8, gpc], f32)
            nc.sync.dma_start(gtmp[:], gt_in[:])
            nc.vector.tensor_copy(gt_s[:], gtmp[:])

            def load_small(ap_in, p, q_, tagn):
                tt = hp.tile([p, q_], f32, tag=tagn + "f")
                nc.sync.dma_start(tt[:], ap_in)
                rr = hp.tile([p, q_], f32r, tag=tagn)
                nc.vector.tensor_copy(rr[:], tt[:])
                return rr

            cur = gt_s
            for i in range(3):
                wmi = load_small(wm[i][:], [8, NG16, NG16][i], NG16, f"wm{i}")
                bmi = hp.tile([NG16, 1], f32, tag=f"bm{i}")
                nc.sync.dma_start(bmi[:], bm[i][:])
                zz = psC.tile([NG16, gpc], f32, tag="qdst", space="PSUM")
                nc.tensor.matmul(zz[:], wmi[:], cur[:], start=True, stop=True)
                nxt = hp.tile([NG16, gpc], f32r, tag=f"m{i}")
                nc.scalar.activation(nxt[:], zz[:], AF.Relu, bias=bmi[:, 0:1])
                cur = nxt

            wr0a = load_small(wr[0][0:128, :], 128, NG16, "wr0a")
            wr0b = load_small(wr[0][128:256, :], 128, NG16, "wr0b")
            wr0c = load_small(wr[0][256:REGIN, :], NG16, NG16, "wr0c")
            z1 = psC.tile([NG16, gpc], f32, tag="qdst", space="PSUM")
            nc.tensor.matmul(z1[:], wr0a[:], poolT[:, 0:gpc], start=True, stop=False)
            nc.tensor.matmul(z1[:], wr0b[:], poolT[:, gpc:2 * gpc], start=False, stop=False)
            nc.tensor.matmul(z1[:], wr0c[:], cur[:], start=False, stop=True)
            br0_s = hp.tile([NG16, 1], f32)
            nc.sync.dma_start(br0_s[:], br[0][:])
            z1s = hp.tile([NG16, gpc], f32r)
            nc.scalar.activation(z1s[:], z1[:], AF.Relu, bias=br0_s[:, 0:1])

            wr1s = load_small(wr[1][:], NG16, NG16, "wr1")
            z2 = psC.tile([NG16, gpc], f32, tag="qdst", space="PSUM")
            nc.tensor.matmul(z2[:], wr1s[:], z1s[:], start=True, stop=True)
            br1_s = hp.tile([NG16, 1], f32)
            nc.sync.dma_start(br1_s[:], br[1][:])
            z2s = hp.tile([NG16, gpc], f32r)
            nc.scalar.activation(z2s[:], z2[:], AF.Relu, bias=br1_s[:, 0:1])

            wr2s = load_small(wr[2][:], NG16, 1, "wr2")
            z3 = psC.tile([1, gpc], f32, tag="qdst", space="PSUM")
            nc.tensor.matmul(z3[:], wr2s[:], z2s[:], start=True, stop=True)
            br2_s = hp.tile([1, 1], f32)
            nc.sync.dma_start(br2_s[:], br[2][:])
            ys = hp.tile([1, gpc], f32)
            nc.scalar.activation(ys[:], z3[:], AF.Identity, bias=br2_s[:, 0:1])
            nc.sync.dma_start(y[:], ys[:])

    nc.compile()
    return nc


# --------------------------------------------------------------------------
# entry point
# --------------------------------------------------------------------------

_LAST = {}


def kernel(x, edge_index, batch, global_features, conv_params, mlp_params,
           reg_params):
    from concourse import bass_utils
    from concourse.bass_interp import get_hw_module

    x = np.asarray(x, np.float32)
    edge_index = np.asarray(edge_index)
    batch_np = np.asarray(batch, np.int64)
    global_features = np.asarray(global_features, np.float32)
    conv_params = [{k: np.asarray(v, np.float32) for k, v in p.items()}
                   for p in conv_params]
    mlp_params = [(np.asarray(W, np.float32), np.asarray(b, np.float32))
                  for W, b in mlp_params]
    reg_params = [(np.asarray(W, np.float32), np.asarray(b, np.float32))
                  for W, b in reg_params]

    n_graphs = global_features.shape[0]
    HC = conv_params[0]["Wq"].shape[1]
    L = len(conv_params)
    D0 = 128
    NG16 = mlp_params[0][0].shape[1]
    gpc = n_graphs // N_CORES

    plan = _plan(x, edge_index, batch_np, n_graphs)
    P_N, NB, T_total = plan["P_N"], plan["NB"], plan["T_total"]

    x_pad = np.zeros((x.shape[0], D0), np.float32)
    x_pad[:, :x.shape[1]] = x
    xls, gpools, gts = _per_core_tables(plan, x_pad, batch_np, global_features,
                                        n_graphs)
    xt = np.concatenate(xls, axis=0)

    shapes = dict(layers=L, D0=D0, HC=HC, NG16=NG16)
    nc = _build(plan, shapes)
    nc.m = get_hw_module(nc.m)

    import ml_dtypes
    # shared (replicated) inputs
    shared = {"xt": xt.astype(ml_dtypes.bfloat16)}
    for l in range(L):
        p = conv_params[l]
        dl = x.shape[1] if l == 0 else HC
        kc = max(1, dl // 128)
        for nm, pair in (("wqs", ("Wq", "Ws")), ("wkv", ("Wk", "Wv"))):
            Wc = np.concatenate([p[pair[0]], p[pair[1]]], axis=1)
            Wfull = np.zeros((kc * 128, 2 * HC), np.float32)
            Wfull[:dl] = Wc
            shared[f"{nm}{l}"] = np.ascontiguousarray(
                Wfull.reshape(kc, 128, 2 * HC).transpose(1, 0, 2).reshape(128, kc * 2 * HC))
        for bn in "qkvs":
            shared[f"b{bn}{l}"] = np.broadcast_to(p["b" + bn], (128, HC)).astype(np.float32).copy()
    for i in range(3):
        W, b = mlp_params[i]
        shared[f"wm{i}"] = W
        shared[f"bm{i}"] = b.reshape(-1, 1)
    for i in range(3):
        W, b = reg_params[i]
        shared[f"wr{i}"] = W
        shared[f"br{i}"] = b.reshape(-1, 1)

    in_maps = []
    for c in range(N_CORES):
        m = dict(shared)
        m["xl"] = xls[c]
        srcs_c, dm_c = plan["metas"][c]
        m["meta"] = srcs_c
        nt = srcs_c.shape[0] // 128
        dmr = dm_c.reshape(nt, 128, 2)
        sm = np.zeros((nt, 128, 128), np.float32)
        ii = np.arange(128)
        for ti in range(nt):
            sm[ti, ii, dmr[ti, :, 0].astype(np.int64)] = dmr[ti, :, 1]
        m["smat"] = sm.reshape(-1, 128).astype(ml_dtypes.bfloat16)
        m["gpool"] = gpools[c]
        m["gt"] = gts[c]
        in_maps.append(m)

    res = bass_utils.run_bass_kernel_spmd(nc, in_maps, core_ids=list(range(N_CORES)))
    out = np.concatenate([res.results[c]["y"][0] for c in range(N_CORES)])
    _LAST.update(nc=nc, in_maps=in_maps, plan=plan)
    return out.astype(np.float32)
